# revision 3
# baseline (speedup 1.0000x reference)
"""RNN-T Joint kernel for Trainium2, 8-core data-parallel over batch.

out[b,t,u,v] = (enc[b,t] + dec[b,u]) @ W.T + bias

Key algebra: the Linear distributes over the broadcast add, so per core we
compute Penc = enc_b @ W.T (256,1024) and Pdec = dec_b @ W.T + bias (64,1024)
on the tensor engine in fp32, then materialize the (16384,1024) output as
out[t*64+u, :] = Penc[t] + Pdec[u] via "selection matmuls": for each pair of
t values, a 0/1 selection matrix M (stationary) against a Combined tensor
holding [Pdec rows | Penc rows] reproduces Penc[t]+Pdec[u] directly in PSUM.
The Combined values are split hi/lo into two bf16 tensors (fp32 = hi + lo to
~2^-17 relative), so the selection matmuls run at bf16 rate (1 cycle/row vs 4
for fp32) while PSUM fp32 accumulation restores near-fp32 precision (~5e-6).
"""

import numpy as np
import ml_dtypes

import concourse.bass as bass
import concourse.mybir as mybir
from concourse.tile import TileContext, ScopedClock, VectorClock
from concourse.tile_scheduler import N_PROCS
from concourse import bass_utils

B, T, U, D, V = 8, 256, 64, 512, 1024
F32 = mybir.dt.float32
BF16 = mybir.dt.bfloat16


class SplitDrainTC(TileContext):
    # The walrus build in this container rejects >1 sync-wait command on the
    # kernel-tail Drain; emit one single-wait nop per logical proc instead.
    def _drain_and_barrier(self, tick_clock, wait_clock):
        gc = tick_clock.global_clock
        for p in range(N_PROCS):
            t = gc[p]
            if t:
                ticks = [0] * N_PROCS
                ticks[p] = t
                nop = self.nc.sync.nop(nofuse=True)
                wait_clock.add_sem_waits(nop.ins, ScopedClock({None: VectorClock(ticks)}))
        self.nc.sync.drain()
        self.nc.all_engine_barrier()
        popped = self.nc._tile_sem_poison_stack.pop()
        assert popped is self._sem_poison
        self.nc.clear_and_free_semaphores(list(self.sems.allocated().values()))
        self.nc.all_engine_barrier()


def _split_multi_waits(nc, max_waits: int = 1):
    # This container's walrus codegen accepts at most one sync-wait command
    # per instruction. Move excess waits onto fresh NoOps inserted just
    # before the instruction in the same block/engine (engine queues are
    # in-order, so waiting on a preceding nop is equivalent).
    k = 0
    for f in nc.m.functions:
        for blk in f.blocks:
            new_list = []
            for inst in blk.instructions:
                si = getattr(inst, "sync_info", None)
                if si is not None and si.on_wait and len(si.on_wait) > max_waits:
                    excess = list(si.on_wait[:-max_waits])
                    si.on_wait = list(si.on_wait[-max_waits:])
                    for w in excess:
                        n = mybir.InstNoOp(name=f"{inst.name}_sw{k}", ins=[], outs=[])
                        k += 1
                        n.engine = inst.engine
                        n.sync_info = mybir.SyncInfo(on_wait=[w], on_update=[])
                        new_list.append(n)
                new_list.append(inst)
            blk.instructions[:] = new_list


def _make_selection_matrices() -> np.ndarray:
    # M[p, j*128+m]: out partition m of tile (g,j) is row (t,u) with
    # t = 64g + 2j + m//64, u = m%64.  Combined[g]: partition u (0..63) holds
    # Pdec[u]; partition 64+l holds Penc[64g+l].
    M = np.zeros((128, 32 * 128), np.float32)
    for j in range(32):
        for m in range(128):
            M[m % 64, j * 128 + m] = 1.0
            M[64 + 2 * j + m // 64, j * 128 + m] = 1.0
    return M.astype(ml_dtypes.bfloat16)


_NC_CACHE = {}


def _build_nc():
    if "nc" in _NC_CACHE:
        return _NC_CACHE["nc"]
    nc = bass.Bass()
    encT_d = nc.dram_tensor("encT", [D, T], F32, kind="ExternalInput")
    decT_d = nc.dram_tensor("decT", [D, U], F32, kind="ExternalInput")
    WT_d = nc.dram_tensor("WT", [D, V], F32, kind="ExternalInput")
    b_d = nc.dram_tensor("bvec", [1, V], F32, kind="ExternalInput")
    out_d = nc.dram_tensor("out", [T * U, V], F32, kind="ExternalOutput")
    M_d = nc.inline_tensor(_make_selection_matrices(), name="selmat")

    KC = D // 128  # 4 contraction chunks

    with SplitDrainTC(nc) as tc:
        with (
            tc.tile_pool(name="const", bufs=1) as cpool,
            tc.tile_pool(name="tmp", bufs=2) as tpool,
            tc.tile_pool(name="outp", bufs=4) as opool,
            tc.tile_pool(name="ps", bufs=4, space="PSUM") as pspool,
        ):
            encT_sb = cpool.tile([128, KC, T], F32, name="encT_sb")
            decT_sb = cpool.tile([128, KC, U], F32, name="decT_sb")
            WT_sb = cpool.tile([128, KC, V], F32, name="WT_sb")
            b_sb = cpool.tile([1, V], F32, name="b_sb")
            ones_sb = cpool.tile([1, U], F32, name="ones_sb")
            M_sb = cpool.tile([128, 32, 128], BF16, name="M_sb")

            nc.sync.dma_start(out=encT_sb, in_=encT_d[:, :].rearrange("(k p) t -> p k t", p=128))
            nc.sync.dma_start(out=decT_sb, in_=decT_d[:, :].rearrange("(k p) u -> p k u", p=128))
            nc.sync.dma_start(out=WT_sb, in_=WT_d[:, :].rearrange("(k p) v -> p k v", p=128))
            nc.sync.dma_start(out=b_sb, in_=b_d[:, :])
            nc.sync.dma_start(out=M_sb, in_=M_d[:, :].rearrange("p (j m) -> p j m", m=128))
            nc.vector.memset(ones_sb, 1.0)

            C_hi = [cpool.tile([128, V], BF16, name=f"c_hi_{g}") for g in range(4)]
            C_lo = [cpool.tile([128, V], BF16, name=f"c_lo_{g}") for g in range(4)]

            # ---- dec projection + bias -> psum partitions 0..63 ----
            ps_dec = pspool.tile([128, V], F32, name="ps")
            for h in range(2):
                sl = slice(h * 512, (h + 1) * 512)
                for kc in range(KC):
                    nc.tensor.matmul(
                        ps_dec[0:64, sl],
                        lhsT=decT_sb[:, kc, :],
                        rhs=WT_sb[:, kc, sl],
                        start=(kc == 0),
                        stop=False,
                    )
                nc.tensor.matmul(
                    ps_dec[0:64, sl], lhsT=ones_sb, rhs=b_sb[:, sl],
                    start=False, stop=True,
                )
            hi32d = tpool.tile([128, V], F32, name="hi32")
            nc.scalar.copy(out=C_hi[0][0:64, :], in_=ps_dec[0:64, :])
            nc.vector.tensor_copy(out=hi32d[0:64, :], in_=C_hi[0][0:64, :])
            nc.vector.tensor_sub(out=C_lo[0][0:64, :], in0=ps_dec[0:64, :], in1=hi32d[0:64, :])
            for g in range(1, 4):
                nc.scalar.copy(out=C_hi[g][0:64, :], in_=C_hi[0][0:64, :])
                nc.vector.tensor_copy(out=C_lo[g][0:64, :], in_=C_lo[0][0:64, :])

            # ---- enc projections -> psum partitions 64..127, one per group ----
            for g in range(4):
                ps_enc = pspool.tile([128, V], F32, name="ps")
                for h in range(2):
                    sl = slice(h * 512, (h + 1) * 512)
                    for kc in range(KC):
                        nc.tensor.matmul(
                            ps_enc[64:128, sl],
                            lhsT=encT_sb[:, kc, 64 * g:64 * (g + 1)],
                            rhs=WT_sb[:, kc, sl],
                            start=(kc == 0),
                            stop=(kc == KC - 1),
                        )
                hi32e = tpool.tile([128, V], F32, name="hi32")
                nc.scalar.copy(out=C_hi[g][64:128, :], in_=ps_enc[64:128, :])
                nc.vector.tensor_copy(out=hi32e[64:128, :], in_=C_hi[g][64:128, :])
                nc.vector.tensor_sub(out=C_lo[g][64:128, :], in0=ps_enc[64:128, :], in1=hi32e[64:128, :])

            # ---- main loop: 128 output tiles of (128 rows, 1024) ----
            for i in range(128):
                g, j = i // 32, i % 32
                ps_t = pspool.tile([128, V], F32, name="ps")
                for h in range(2):
                    sl = slice(h * 512, (h + 1) * 512)
                    nc.tensor.matmul(
                        ps_t[:, sl], lhsT=M_sb[:, j, :], rhs=C_hi[g][:, sl],
                        start=True, stop=False,
                    )
                    nc.tensor.matmul(
                        ps_t[:, sl], lhsT=M_sb[:, j, :], rhs=C_lo[g][:, sl],
                        start=False, stop=True,
                    )
                out_t = opool.tile([128, V], F32, name="out_t")
                if i % 2 == 0:
                    nc.scalar.copy(out=out_t, in_=ps_t)
                else:
                    nc.vector.tensor_copy(out=out_t, in_=ps_t)
                nc.sync.dma_start(out=out_d[128 * i:128 * (i + 1), :], in_=out_t)

    _split_multi_waits(nc)
    _NC_CACHE["nc"] = nc
    return nc


def kernel(encoder_outputs: np.ndarray, decoder_outputs: np.ndarray,
           W: np.ndarray, b: np.ndarray, **_ignored) -> np.ndarray:
    nc = _build_nc()
    WT = np.ascontiguousarray(W.T.astype(np.float32))
    bvec = np.ascontiguousarray(b.astype(np.float32).reshape(1, V))
    in_maps = []
    for core in range(B):
        in_maps.append({
            "encT": np.ascontiguousarray(encoder_outputs[core].T.astype(np.float32)),
            "decT": np.ascontiguousarray(decoder_outputs[core].T.astype(np.float32)),
            "WT": WT,
            "bvec": bvec,
        })
    res = bass_utils.run_bass_kernel_spmd(nc, in_maps, core_ids=list(range(B)))
    _NC_CACHE["last_result"] = res
    out = np.stack([res.results[c]["out"].reshape(T, U, V) for c in range(B)])
    return out
